# revision 1
# baseline (speedup 1.0000x reference)
"""GNN neighbor-max kernel — ap_gather d=8 channel-block design.

Per core: 2 samples, batch-parallel across the 8 NeuronCores. Per sample:
  table xe[16g+q, n, j] = x[8q+j, n]  (one full copy per GPSIMD group,
  128KB/partition) so ONE gather index fetches all 128 channels of a node.
  Group g owns nodes [g*512, (g+1)*512); its index list packs 17 slots per
  node (16 neighbors + the node itself, folding the final self-max into the
  K-reduce): I_g[n_local*17 + k].
  16 chunks x 544 idx: ap_gather -> gt [128, 544*8] (32 nodes/group/chunk),
  DVE reduce over k=17 (strided 4D view) -> oblk[128, 8, 512],
  then 8 per-group DMAs write oblk back to out[C, N] row-major.
"""

import os

import numpy as np

import concourse.bacc as bacc
import concourse.bass as bass
import concourse.mybir as mybir
from concourse.bass_utils import run_bass_kernel_spmd

B, C, N, K = 16, 128, 4096, 16
N_CORES = 8
S = B // N_CORES
D = 8                      # channels per partition block
NG = 8                     # gpsimd groups
NODES_PER_GROUP = N // NG  # 512
CHUNKS = 16
KS = K + 1                            # 16 neighbors + self
NODES_PER_CHUNK = NODES_PER_GROUP // CHUNKS  # 32 nodes per group per chunk
NI = NODES_PER_CHUNK * KS            # 544 idx per gather
PIPELINED = os.environ.get("PIPELINE", "1") == "1"

_NC_CACHE = {}


def _build_program():
    nc = bacc.Bacc(None, target_bir_lowering=False)

    ncols = N // NG * KS // 16  # 544 idx columns per sample
    xe_d = nc.dram_tensor("xe", [S, C, N * D], mybir.dt.float32, kind="ExternalInput")
    idx_d = nc.dram_tensor("idx", [S, C, ncols], mybir.dt.int16,
                           kind="ExternalInput")
    out_d = nc.dram_tensor("out", [S, C, N], mybir.dt.float32, kind="ExternalOutput")

    with (
        nc.Block() as block,
        nc.semaphore("dsem") as dsem,   # input DMAs
        nc.semaphore("gsem") as gsem,   # gather chunks done
        nc.semaphore("vsem") as vsem,   # reduces done
        nc.semaphore("msem") as msem,   # per-sample maxes done
        nc.semaphore("osem") as osem,   # out DMAs done
        nc.sbuf_tensor("tbl", [C, N * D], mybir.dt.float32) as tbl,          # 128KB/p
        nc.sbuf_tensor("gt0", [C, NI * D], mybir.dt.float32) as gt0,         # 16KB/p
        nc.sbuf_tensor("gt1", [C, NI * D], mybir.dt.float32) as gt1,         # 16KB/p
        nc.sbuf_tensor("oblk", [C, D * N // NG], mybir.dt.float32) as oblk,  # 16KB/p
        nc.sbuf_tensor("idxt", [C, S * (N // NG) * KS // 16], mybir.dt.int16) as idxt,
        nc.sbuf_tensor("msem_probe", [1, 4], mybir.dt.float32) as msem_probe,
    ):
        gts = [gt0, gt1]

        @block.sync
        def _(sy: bass.BassEngine):
            # all idx up front (small)
            for s in range(S):
                sy.dma_start(out=idxt[:, s * ncols:(s + 1) * ncols],
                             in_=idx_d[s]).then_inc(dsem, 16)
            for s in range(S):
                if s > 0:
                    # table buffer reused: only the gathers read tbl, so it can
                    # be overwritten as soon as sample s-1's gathers retire
                    sy.wait_ge(gsem, CHUNKS * s)
                sy.dma_start(out=tbl[:], in_=xe_d[s]).then_inc(dsem, 16)

        @block.gpsimd
        def _(g: bass.BassGpSimd):
            for s in range(S):
                g.wait_ge(dsem, 16 * S + 16 * (s + 1))  # all idx + table s
                for c in range(CHUNKS):
                    ci = s * CHUNKS + c
                    if ci >= 2:
                        back = 1 if PIPELINED else 0
                        g.wait_ge(vsem, ci - back)
                    col0 = s * ncols + c * (NI // 16)
                    g.ap_gather(
                        out_ap=gts[ci % 2][:],
                        in_ap=tbl[:],
                        idxs_ap=idxt[:, col0:col0 + NI // 16],
                        channels=C, num_elems=N, d=D, num_idxs=NI,
                    ).then_inc(gsem, 1)

        @block.vector
        def _(v: bass.BassVectorEngine):
            for s in range(S):
                for c in range(CHUNKS):
                    ci = s * CHUNKS + c
                    v.wait_ge(gsem, ci + 1)
                    if s > 0 and c == 0:
                        v.wait_ge(osem, 256 * s)  # oblk drained (2x8 DMAs x16)
                    gt = gts[ci % 2]
                    gin = gt[:].rearrange("p (n k j) -> p j n k", k=KS, j=D)
                    oout = oblk[:].rearrange("p (j n) -> p j n", j=D)[
                        :, :, c * NODES_PER_CHUNK:(c + 1) * NODES_PER_CHUNK]
                    v.tensor_reduce(out=oout, in_=gin,
                                    axis=mybir.AxisListType.X,
                                    op=mybir.AluOpType.max).then_inc(vsem, 1)
                    if c == CHUNKS // 2 - 1:
                        # first half of oblk final -> early out-DMAs
                        v.wait_ge(vsem, s * CHUNKS + CHUNKS // 2)
                        v.memset(msem_probe[:1, :1], 0).then_inc(msem, 1)
                # all reduces done -> sample complete (self folded into gather)
                v.wait_ge(vsem, (s + 1) * CHUNKS)
                v.memset(msem_probe[:1, :1], 0).then_inc(msem, 1)

        @block.scalar
        def _(sc: bass.BassEngine):
            half = NODES_PER_GROUP // 2
            for s in range(S):
                for h in range(2):
                    sc.wait_ge(msem, 2 * s + h + 1)
                    for gg in range(NG):
                        src = oblk[gg * 16:(gg + 1) * 16].rearrange(
                            "p (j n) -> p j n", j=D)[:, :, h * half:(h + 1) * half]
                        dst = bass.AP(
                            out_d,
                            s * C * N + gg * NODES_PER_GROUP + h * half,
                            [[D * N, 16], [N, D], [1, half]],
                        )
                        sc.dma_start(out=dst, in_=src).then_inc(osem, 16)

    nc.compile()
    return nc


def _prep_sample(x_s: np.ndarray, nidx_s: np.ndarray):
    """x_s [C, N] f32, nidx_s [N, K] int -> (xe [C, N*D] f32, idx [C, N*K/16] i16)."""
    xq = x_s.reshape(16, D, N).transpose(0, 2, 1)          # [q, n, j]
    xe = np.broadcast_to(xq[None], (NG, 16, N, D)).reshape(C, N * D)
    idx16 = np.ascontiguousarray(nidx_s, dtype=np.int16)   # [N, K]
    self_col = np.arange(N, dtype=np.int16)[:, None]       # [N, 1]
    idx17 = np.concatenate([idx16, self_col], axis=1)      # [N, 17]
    blocks = []
    for g in range(NG):
        flat = idx17[g * NODES_PER_GROUP:(g + 1) * NODES_PER_GROUP].reshape(-1)
        blocks.append(flat.reshape(-1, 16).T)              # [16, N*KS/16/NG]
    idx = np.concatenate(blocks, axis=0)                   # [128, 544]
    return np.ascontiguousarray(xe), np.ascontiguousarray(idx)


def _run(x: np.ndarray, neighbor_idx: np.ndarray, **spmd_kwargs):
    x = np.asarray(x, dtype=np.float32)
    neighbor_idx = np.asarray(neighbor_idx)

    if "nc" not in _NC_CACHE:
        _NC_CACHE["nc"] = _build_program()
    nc = _NC_CACHE["nc"]

    in_maps = []
    for core in range(N_CORES):
        lo = core * S
        xes, idxs = [], []
        for s in range(S):
            xe, idx = _prep_sample(x[lo + s], neighbor_idx[lo + s])
            xes.append(xe)
            idxs.append(idx)
        in_maps.append({
            "xe": np.stack(xes, axis=0),
            "idx": np.stack(idxs, axis=0),
        })

    res = run_bass_kernel_spmd(nc, in_maps, core_ids=list(range(N_CORES)),
                               **spmd_kwargs)
    out = np.concatenate([res.results[core]["out"] for core in range(N_CORES)],
                         axis=0)
    return out.astype(np.float32), res


def kernel(x: np.ndarray, neighbor_idx: np.ndarray) -> np.ndarray:
    return _run(x, neighbor_idx)[0]


if __name__ == "__main__":
    rng = np.random.default_rng(0)
    xt = rng.standard_normal((B, C, N)).astype(np.float32)
    it = rng.integers(0, N, size=(B, N, K)).astype(np.int64)
    got = kernel(xt, it)
    ref = np.maximum(
        np.max(xt[np.arange(B)[:, None, None], :, it], axis=2).transpose(0, 2, 1),
        xt,
    )
    print("abs err:", np.abs(got - ref).max())



# revision 2
# speedup vs baseline: 1.2405x; 1.2405x over previous
"""GNN neighbor-max kernel — bf16 ap_gather + pair-max-tree design.

Per core: 2 samples, batch-parallel across the 8 NeuronCores. Per sample:
  bf16 table xe[16g+q, m, j] = x[8q+j, (m + 512g) % N]  (one copy per GPSIMD
  group, rolled by the group's node base so each group's own nodes sit at
  uniform offsets; 64KB/partition, double-buffered across samples).
  Group g owns nodes [g*512, (g+1)*512); its per-chunk index list holds the
  16 neighbors of 32 nodes, pre-shifted by -512g mod N: one ap_gather of 512
  idx -> gt [128, 512*8] bf16.
  DVE reduces k=16 via a contiguous pair-max tree (16->8->4->2->1, 2-byte
  packed innermost so the DVE 2x mode applies), then a final max against the
  table's own x slice (self node, uniform offset thanks to the roll) writes
  transposed into oblk [128, (j, n)].
  Per-sample oblk buffers; scalar engine drains halves to out[C, N] bf16.
"""

import numpy as np
import ml_dtypes

import concourse.bacc as bacc
import concourse.bass as bass
import concourse.mybir as mybir
from concourse.bass_utils import run_bass_kernel_spmd

B, C, N, K = 16, 128, 4096, 16
N_CORES = 8
S = B // N_CORES
D = 8                      # channels per partition block
NG = 8                     # gpsimd groups
NPG = N // NG              # 512 nodes per group
CHUNKS = 16
NPC = NPG // CHUNKS        # 32 nodes per group per chunk
NI = NPC * K               # 512 idx per gather
NCOLS = NPG * K // 16      # 512 idx columns per sample per partition

_NC_CACHE = {}


def _build_program():
    nc = bacc.Bacc(None, target_bir_lowering=False)

    bf16 = mybir.dt.bfloat16
    xe_d = nc.dram_tensor("xe", [S, C, N * D], bf16, kind="ExternalInput")
    idx_d = nc.dram_tensor("idx", [S, C, NCOLS], mybir.dt.int16,
                           kind="ExternalInput")
    out_d = nc.dram_tensor("out", [S, C, N], bf16, kind="ExternalOutput")

    with (
        nc.Block() as block,
        nc.semaphore("isem") as isem,   # idx DMAs
        nc.semaphore("tsem") as tsem,   # table DMAs
        nc.semaphore("gsem") as gsem,   # gather chunks done
        nc.semaphore("bsem") as bsem,   # tree level A done (gt free)
        nc.semaphore("msem") as msem,   # per-half-sample maxes done
        nc.semaphore("osem") as osem,   # out DMAs done
        nc.sbuf_tensor("tbl0", [C, N * D], bf16) as tbl0,      # 64KB/p
        nc.sbuf_tensor("tbl1", [C, N * D], bf16) as tbl1,      # 64KB/p
        nc.sbuf_tensor("gt0", [C, NI * D], bf16) as gt0,       # 8KB/p
        nc.sbuf_tensor("gt1", [C, NI * D], bf16) as gt1,
        nc.sbuf_tensor("gt2", [C, NI * D], bf16) as gt2,
        nc.sbuf_tensor("gt3", [C, NI * D], bf16) as gt3,
        nc.sbuf_tensor("tA", [C, NPC * 8 * D], bf16) as tA,    # 4KB/p
        nc.sbuf_tensor("tB", [C, NPC * 4 * D], bf16) as tB,
        nc.sbuf_tensor("tC", [C, NPC * 2 * D], bf16) as tC,
        nc.sbuf_tensor("tD", [C, NPC * D], bf16) as tD,
        nc.sbuf_tensor("ob0", [C, D * NPG], bf16) as ob0,      # 8KB/p
        nc.sbuf_tensor("ob1", [C, D * NPG], bf16) as ob1,
        nc.sbuf_tensor("idxt", [C, S * NCOLS], mybir.dt.int16) as idxt,
    ):
        tbls = [tbl0, tbl1]
        gts = [gt0, gt1, gt2, gt3]
        obs = [ob0, ob1]

        @block.sync
        def _(sy: bass.BassEngine):
            for s in range(S):
                sy.dma_start(out=idxt[:, s * NCOLS:(s + 1) * NCOLS],
                             in_=idx_d[s]).then_inc(isem, 16)
            for s in range(S):
                sy.dma_start(out=tbls[s][:], in_=xe_d[s]).then_inc(tsem, 16)

        @block.gpsimd
        def _(g: bass.BassGpSimd):
            g.wait_ge(isem, 16 * S)
            for s in range(S):
                g.wait_ge(tsem, 16 * (s + 1))
                for c in range(CHUNKS):
                    ci = s * CHUNKS + c
                    if ci >= 4:
                        g.wait_ge(bsem, ci - 3)
                    col0 = s * NCOLS + c * (NI // 16)
                    g.ap_gather(
                        out_ap=gts[ci % 4][:],
                        in_ap=tbls[s][:],
                        idxs_ap=idxt[:, col0:col0 + NI // 16],
                        channels=C, num_elems=N, d=D, num_idxs=NI,
                    ).then_inc(gsem, 1)

        @block.vector
        def _(v: bass.BassVectorEngine):
            for s in range(S):
                for c in range(CHUNKS):
                    ci = s * CHUNKS + c
                    v.wait_ge(gsem, ci + 1)
                    gv = gts[ci % 4][:].rearrange("p (n k j) -> p n k j",
                                                  k=K, j=D)
                    av = tA[:].rearrange("p (n t j) -> p n t j", t=8, j=D)
                    bv = tB[:].rearrange("p (n t j) -> p n t j", t=4, j=D)
                    cv = tC[:].rearrange("p (n t j) -> p n t j", t=2, j=D)
                    dv = tD[:].rearrange("p (n j) -> p n j", j=D)
                    v.tensor_max(out=av, in0=gv[:, :, 0:K:2, :],
                                 in1=gv[:, :, 1:K:2, :]).then_inc(bsem, 1)
                    v.tensor_max(out=bv, in0=av[:, :, 0:8:2, :],
                                 in1=av[:, :, 1:8:2, :])
                    v.tensor_max(out=cv, in0=bv[:, :, 0:4:2, :],
                                 in1=bv[:, :, 1:4:2, :])
                    v.tensor_max(out=dv, in0=cv[:, :, 0, :],
                                 in1=cv[:, :, 1, :])
                    sv = tbls[s][:].rearrange("p (n j) -> p n j", j=D)[
                        :, c * NPC:(c + 1) * NPC, :]
                    ov = obs[s][:].rearrange("p (j n) -> p j n", j=D)
                    ov = ov.transpose([0, 2, 1])[:, c * NPC:(c + 1) * NPC, :]
                    e = v.tensor_max(out=ov, in0=dv, in1=sv)
                    if c == CHUNKS // 2 - 1 or c == CHUNKS - 1:
                        e.then_inc(msem, 1)

        @block.scalar
        def _(sc: bass.BassEngine):
            half = NPG // 2
            for s in range(S):
                for h in range(2):
                    sc.wait_ge(msem, 2 * s + h + 1)
                    for gg in range(NG):
                        src = obs[s][gg * 16:(gg + 1) * 16].rearrange(
                            "p (j n) -> p j n", j=D)[:, :, h * half:(h + 1) * half]
                        dst = bass.AP(
                            out_d,
                            s * C * N + gg * NPG + h * half,
                            [[D * N, 16], [N, D], [1, half]],
                        )
                        sc.dma_start(out=dst, in_=src).then_inc(osem, 16)

    nc.compile()
    return nc


def _prep_sample(x_s: np.ndarray, nidx_s: np.ndarray):
    """x_s [C, N] f32, nidx_s [N, K] int -> (xe [C, N*D] bf16, idx [C, NCOLS] i16)."""
    xq = x_s.reshape(16, D, N).transpose(0, 2, 1)          # [q, n, j]
    xe = np.empty((NG, 16, N, D), dtype=np.float32)
    for g in range(NG):
        xe[g] = np.roll(xq, -NPG * g, axis=1)              # group-rolled copy
    xe = xe.reshape(C, N * D).astype(ml_dtypes.bfloat16)
    nidx = np.asarray(nidx_s, dtype=np.int64)              # [N, K]
    blocks = []
    for g in range(NG):
        blk = (nidx[g * NPG:(g + 1) * NPG] - NPG * g) % N  # [512, 16]
        flat = blk.reshape(-1).astype(np.int16)            # node-major
        blocks.append(flat.reshape(-1, 16).T)              # [16, 512]
    idx = np.concatenate(blocks, axis=0)                   # [128, 512]
    return np.ascontiguousarray(xe), np.ascontiguousarray(idx)


def _run(x: np.ndarray, neighbor_idx: np.ndarray, **spmd_kwargs):
    x = np.asarray(x, dtype=np.float32)
    neighbor_idx = np.asarray(neighbor_idx)

    if "nc" not in _NC_CACHE:
        _NC_CACHE["nc"] = _build_program()
    nc = _NC_CACHE["nc"]

    in_maps = []
    for core in range(N_CORES):
        lo = core * S
        xes, idxs = [], []
        for s in range(S):
            xe, idx = _prep_sample(x[lo + s], neighbor_idx[lo + s])
            xes.append(xe)
            idxs.append(idx)
        in_maps.append({
            "xe": np.stack(xes, axis=0),
            "idx": np.stack(idxs, axis=0),
        })

    res = run_bass_kernel_spmd(nc, in_maps, core_ids=list(range(N_CORES)),
                               **spmd_kwargs)
    out = np.concatenate([res.results[core]["out"] for core in range(N_CORES)],
                         axis=0)
    return out.astype(np.float32), res


def kernel(x: np.ndarray, neighbor_idx: np.ndarray) -> np.ndarray:
    return _run(x, neighbor_idx)[0]


if __name__ == "__main__":
    rng = np.random.default_rng(0)
    xt = rng.standard_normal((B, C, N)).astype(np.float32)
    it = rng.integers(0, N, size=(B, N, K)).astype(np.int64)
    got = kernel(xt, it)
    ref = np.maximum(
        np.max(xt[np.arange(B)[:, None, None], :, it], axis=2).transpose(0, 2, 1),
        xt,
    )
    xb = xt.astype(ml_dtypes.bfloat16).astype(np.float32)
    refb = np.maximum(
        np.max(xb[np.arange(B)[:, None, None], :, it], axis=2).transpose(0, 2, 1),
        xb,
    )
    print("abs err vs f32 ref:", np.abs(got - ref).max())
    print("abs err vs bf16 ref:", np.abs(got - refb).max())
